# revision 3
# baseline (speedup 1.0000x reference)
"""TRN2 Bass kernel for nn_KVGather: out[b,i,t] = kv[b, r_idx[b,i,t]] * r_weight[b,i,t].

Full shapes: r_idx/r_weight (32,49,4), kv (32,49,64,256) f32 -> out (32,49,4,64,256) f32.

Sharding: batch dim n=32 across 8 cores (4 batches/core), pure data parallel.

Per-core device kernel:
  - KV shard (196 rows x 16384 f32) resident in SBUF as [128p, 196*128 f32]
    (partition p holds f32 elements [p*128, (p+1)*128) of each row).
  - Host passes per-output-tile SBUF row offsets (int32, = row*128) and a
    [128, 784] broadcast weight matrix. These are runtime data, the program
    is input-independent.
  - For each group of 16 output tiles: batched reg_load of the offsets on
    DVE and ACT, then per tile one gather+scale op ([128,128] f32, dynamic
    start AP) into a staging tile; one 1MB DMA per group to DRAM.
"""

import os
import sys

sys.path.insert(0, "/opt/trn_rl_repo")

import numpy as np

N, P2, TOPK, HW_KV, C_KV = 32, 49, 4, 64, 256
NCORES = 8
NB = N // NCORES  # batches per core
ROWS = NB * P2  # kv rows per core
TILES = NB * P2 * TOPK  # output tiles per core
ROW_ELEMS = HW_KV * C_KV  # 16384 f32 per row/tile
PPART = 128
CROW = ROW_ELEMS // PPART  # 128 f32 per partition per row
GROUP = 16  # output tiles per staging buffer
NGROUPS = TILES // GROUP  # 49

# tile j goes to ACT when j % 4 == 3, else DVE (2x-mode DVE is ~2-3x faster)
def _is_act(j):
    return j % 4 == 3


_compiled = None


def _build():
    import concourse.bass as bass
    import concourse.tile as tile
    from concourse import bacc, mybir

    nc = bacc.Bacc("TRN2", target_bir_lowering=False, debug=False)

    f32 = mybir.dt.float32
    i32 = mybir.dt.int32

    n_act = sum(1 for j in range(TILES) if _is_act(j))
    n_dve = TILES - n_act

    kv_d = nc.dram_tensor("kv", [ROWS, ROW_ELEMS], f32, kind="ExternalInput").ap()
    offs_dve_d = nc.dram_tensor("offs_dve", [1, n_dve], i32, kind="ExternalInput").ap()
    offs_act_d = nc.dram_tensor("offs_act", [1, n_act], i32, kind="ExternalInput").ap()
    wq_d = nc.dram_tensor("wq", [PPART, TILES], f32, kind="ExternalInput").ap()
    out_d = nc.dram_tensor("out", [TILES, ROW_ELEMS], f32, kind="ExternalOutput").ap()

    DVE = mybir.EngineType.DVE
    ACT = mybir.EngineType.Activation
    COPY = mybir.ActivationFunctionType.Copy
    MAX_OFF = (ROWS - 1) * CROW

    with tile.TileContext(nc) as tc:
        with (
            tc.tile_pool(name="resident", bufs=1) as res_pool,
            tc.tile_pool(name="stage", bufs=4) as stage_pool,
        ):
            kv_sb = res_pool.tile([PPART, ROWS * CROW], f32, tag="kv")
            offs_dve_sb = res_pool.tile([1, n_dve], i32, tag="offs_dve")
            offs_act_sb = res_pool.tile([1, n_act], i32, tag="offs_act")
            wq_sb = res_pool.tile([PPART, TILES], f32, tag="wq")

            nc.sync.dma_start(offs_dve_sb[:], offs_dve_d[:])
            nc.sync.dma_start(offs_act_sb[:], offs_act_d[:])
            nc.sync.dma_start(wq_sb[:], wq_d[:])

            # kv load: kv_sb[p, r*128 + c] = kv[r, p*128 + c]; 4 chunks of 49 rows
            kv_src = kv_d.rearrange("r (p c) -> p r c", p=PPART)
            kv_dst = kv_sb[:].rearrange("p (r c) -> p r c", c=CROW)
            rchunk = ROWS // 4
            for q in range(4):
                nc.sync.dma_start(
                    kv_dst[:, q * rchunk : (q + 1) * rchunk, :],
                    kv_src[:, q * rchunk : (q + 1) * rchunk, :],
                )

            out_v = out_d.rearrange("(g jj) (p c) -> g p jj c", jj=GROUP, p=PPART)

            dpos = apos = 0
            for g in range(NGROUPS):
                js = list(range(g * GROUP, (g + 1) * GROUP))
                g_dve = [j for j in js if not _is_act(j)]
                g_act = [j for j in js if _is_act(j)]

                _, dve_vals = nc.values_load_multi_w_load_instructions(
                    offs_dve_sb[0:1, dpos : dpos + len(g_dve)],
                    engines=[DVE],
                    min_val=0,
                    max_val=MAX_OFF,
                    skip_runtime_bounds_check=True,
                )
                _, act_vals = nc.values_load_multi_w_load_instructions(
                    offs_act_sb[0:1, apos : apos + len(g_act)],
                    engines=[ACT],
                    min_val=0,
                    max_val=MAX_OFF,
                    skip_runtime_bounds_check=True,
                )
                dpos += len(g_dve)
                apos += len(g_act)
                dve_it = iter(dve_vals)
                act_it = iter(act_vals)

                stage = stage_pool.tile([PPART, GROUP * CROW], f32, tag="stage")
                for k, j in enumerate(js):
                    dst = stage[:, k * CROW : (k + 1) * CROW]
                    scale = wq_sb[:, j : j + 1]
                    if _is_act(j):
                        src = kv_sb[:, bass.ds(next(act_it), CROW)]
                        nc.scalar.activation(dst, src, COPY, scale=scale)
                    else:
                        src = kv_sb[:, bass.ds(next(dve_it), CROW)]
                        nc.vector.tensor_scalar(
                            dst, src, scale, None, mybir.AluOpType.mult
                        )

                nc.sync.dma_start(
                    out_v[g],
                    stage[:].rearrange("p (jj c) -> p jj c", c=CROW),
                )

    nc.compile()
    return nc


def _get_compiled():
    global _compiled
    if _compiled is None:
        _compiled = _build()
    return _compiled


def _enable_trace_hook():
    """Register the axon NTFF profile hook (missing antenv.axon_hooks shim)."""
    import types

    try:
        import antenv.axon_hooks  # noqa: F401

        return
    except ImportError:
        pass
    try:
        import antenv

        mod = types.ModuleType("antenv.axon_hooks")
        holder = {}
        mod.set_axon_ntff_profile_hook = lambda h: holder.__setitem__("h", h)
        mod.get_axon_ntff_profile_hook = lambda: holder.get("h")
        antenv.axon_hooks = mod
        sys.modules["antenv.axon_hooks"] = mod
        if "/root/.axon_site" not in sys.path:
            sys.path.insert(0, "/root/.axon_site")
        from trn_agent_boot.trn_boot import _ntff_profile_via_ctypes

        mod.set_axon_ntff_profile_hook(
            _ntff_profile_via_ctypes("/opt/axon/libaxon_pjrt.so")
        )

        import concourse.bass_utils as bu

        orig = bu.upload_artifacts

        def _safe_upload(tmpdir):
            try:
                return orig(tmpdir)
            except Exception:
                return tmpdir

        bu.upload_artifacts = _safe_upload
    except Exception as e:  # tracing is best-effort
        print(f"trace hook setup failed: {e}")


def kernel(r_idx, r_weight, kv):
    from concourse.bass_utils import run_bass_kernel_spmd

    r_idx = np.asarray(r_idx)
    r_weight = np.asarray(r_weight, dtype=np.float32)
    kv = np.ascontiguousarray(np.asarray(kv, dtype=np.float32))
    assert r_idx.shape == (N, P2, TOPK) and kv.shape == (N, P2, HW_KV, C_KV)

    nc = _get_compiled()

    dve_js = [j for j in range(TILES) if not _is_act(j)]
    act_js = [j for j in range(TILES) if _is_act(j)]

    in_maps = []
    for c in range(NCORES):
        b0 = c * NB
        kv_shard = kv[b0 : b0 + NB].reshape(ROWS, ROW_ELEMS)
        idx_shard = r_idx[b0 : b0 + NB].astype(np.int64)  # (NB, P2, TOPK)
        rows = (np.arange(NB)[:, None, None] * P2 + idx_shard).reshape(-1)
        offs = (rows * CROW).astype(np.int32)  # per-tile SBUF element offset
        w_flat = r_weight[b0 : b0 + NB].reshape(-1).astype(np.float32)
        wq = np.ascontiguousarray(np.broadcast_to(w_flat, (PPART, TILES)))
        in_maps.append(
            {
                "kv": kv_shard,
                "offs_dve": np.ascontiguousarray(offs[dve_js][None, :]),
                "offs_act": np.ascontiguousarray(offs[act_js][None, :]),
                "wq": wq,
            }
        )

    trace = bool(int(os.environ.get("KV_TRACE", "0")))
    if trace:
        _enable_trace_hook()
    res = run_bass_kernel_spmd(nc, in_maps, list(range(NCORES)), trace=trace)

    if trace:
        kernel.last_exec_time_ns = res.exec_time_ns
        kernel.last_trace = (
            res.instructions_and_trace[1] if res.instructions_and_trace else None
        )

    out = np.empty((N, P2, TOPK, HW_KV, C_KV), dtype=np.float32)
    for c in range(NCORES):
        b0 = c * NB
        out[b0 : b0 + NB] = res.results[c]["out"].reshape(NB, P2, TOPK, HW_KV, C_KV)
    return out
